# revision 13
# baseline (speedup 1.0000x reference)
"""Trainium2 Bass kernel for a dense transformer block (single-head attn + MLP).

v3 design (vs v1 baseline at 651 us):
- No collectives: core c handles batch b=c//2, query-half h=c%2, and
  redundantly computes K,V for ALL 2048 tokens of its batch (the v1 pair
  AllGather measured ~210 us of serial ring time / ~200 us PE idle; the
  duplicate K/V projections cost only ~29 us of PE).
- bf16 matmul operands everywhere (PSUM accumulation stays fp32).  Same PE
  rate as f32r but halves SBUF/DMA, so exp(att) and V stay SBUF-resident
  (no DRAM spill round-trips); K round-trips DRAM (SBUF is tight).
- LN stats matmuls (ones-column trick) run on raw x with no PE dependency
  stalls; x is then normalized IN PLACE (two DVE row-broadcast ops per
  chunk) so every projection eviction is a single cheap DVE op and PSUM
  banks release fast.
- softmax without max subtraction (|logits| < ~3, verified), denominator
  and v-bias folded into the y eviction; gelu+bias evicted directly from
  PSUM on the scalar engine; fast Newton reciprocals.
Host permutes tokens so each core's own query tokens are columns 0..1023
(SPMD uniform program); keys cover all 2048 columns.
"""

import numpy as np
import ml_dtypes
import concourse.bass as bass
import concourse.mybir as mybir
import concourse.tile as tile
from concourse import bacc
from concourse.bass_utils import run_bass_kernel_spmd

F32 = mybir.dt.float32
F32R = mybir.dt.float32r
BF16 = mybir.dt.bfloat16
AF = mybir.ActivationFunctionType
ALU = mybir.AluOpType

P = 128
C = 1024        # n_embd
T = 2048        # key tokens per core (full batch)
TQ = 1024       # query tokens per core
H = 4096        # mlp hidden
CK = C // P     # 8
HK = H // P     # 32
S = T // P      # 16 key tiles
NCH = 512       # matmul moving-dim chunk
EPS = 1e-5
ATT_SCALE = 1.0 / 32.0   # 1/sqrt(C)

N_CORES = 8
BFNP = ml_dtypes.bfloat16


def _build():
    nc = bacc.Bacc()

    xTb = nc.declare_dram_parameter("xTb", [C, T], BF16, isOutput=False)
    xq32 = nc.declare_dram_parameter("xq32", [C, TQ], F32, isOutput=False)
    w1qk = nc.declare_dram_parameter("w1qk", [2 * CK, P, C], BF16,
                                     isOutput=False)
    w1v = nc.declare_dram_parameter("w1v", [CK, P, C], BF16, isOutput=False)
    wp = nc.declare_dram_parameter("wp", [CK, P, C], BF16, isOutput=False)
    w2 = nc.declare_dram_parameter("w2", [HK, P, C], BF16, isOutput=False)
    wm = nc.declare_dram_parameter("wm", [CK, P, H], BF16, isOutput=False)
    c1q = nc.declare_dram_parameter("c1q", [CK, P], F32, isOutput=False)
    c1k = nc.declare_dram_parameter("c1k", [CK, P], F32, isOutput=False)
    c1v = nc.declare_dram_parameter("c1v", [CK, P], F32, isOutput=False)
    bp = nc.declare_dram_parameter("bp", [CK, P], F32, isOutput=False)
    c2 = nc.declare_dram_parameter("c2", [HK, P], F32, isOutput=False)
    bm = nc.declare_dram_parameter("bm", [CK, P], F32, isOutput=False)
    onc = nc.declare_dram_parameter("onc", [P, 1], BF16, isOutput=False)
    onr = nc.declare_dram_parameter("onr", [1, P], F32R, isOutput=False)
    out_t = nc.declare_dram_parameter("out_t", [C, TQ], F32, isOutput=True)

    ktd = nc.dram_tensor("ktd", [CK, P, T], BF16)

    xT3 = xTb.rearrange("(k p) t -> p k t", p=P)
    xq3 = xq32.rearrange("(k p) t -> p k t", p=P)

    with tile.TileContext(nc) as tc:
        with tc.tile_pool(name="gp", bufs=1) as gp:
            ones_col = gp.tile([P, 1], BF16)
            nc.sync.dma_start(ones_col[:], onc[:])
            ones_row = gp.tile([1, P], F32R)
            nc.sync.dma_start(ones_row[:], onr[:])

            def colvec(name, src, w=CK):
                t = gp.tile([P, w], F32, tag=name)
                nc.sync.dma_start(t[:], src.rearrange("j p -> p j"))
                return t

            c1q_t = colvec("c1q", c1q)
            c1k_t = colvec("c1k", c1k)
            c1v_t = colvec("c1v", c1v)
            bp_t = colvec("bp", bp)
            bm_t = colvec("bm", bm)
            c2_t = colvec("c2", c2, HK)
            eps_col = gp.tile([P, 1], F32)
            nc.vector.memset(eps_col[:], EPS)

            r_b = gp.tile([P, T], BF16)
            mu_b = gp.tile([P, T], BF16)
            recip_b = gp.tile([P, TQ], BF16)
            r2_b = gp.tile([P, TQ], BF16)
            mu2_b = gp.tile([P, TQ], BF16)
            x2b = gp.tile([P, CK, TQ], BF16)  # attn-sublayer output (bf16)

            def ln_stats(sbp, pp, src3, width, rb, mub, dst3=None):
                """LN row stats over channels of transposed bf16 activations;
                fills rb = 1/sigma and mub = mu broadcast to all partitions,
                then (if dst3) normalizes chunk-by-chunk:
                dst3 = (src3 - mu) * r (in-place when dst3 is src3)."""
                for sub in range(width // NCH):
                    lo = sub * NCH
                    mu_ps = pp.tile([1, NCH], F32, tag="ps", bufs=6,
                                    name="mu_ps")
                    s2_ps = pp.tile([1, NCH], F32, tag="ps", bufs=6,
                                    name="s2_ps")
                    for k in range(CK):
                        nc.tensor.matmul(mu_ps[:], ones_col[:],
                                         src3[:, k, lo:lo + NCH],
                                         start=(k == 0), stop=(k == CK - 1))
                    for k in range(CK):
                        sq = sbp.tile([P, NCH], BF16, tag="sq", bufs=2)
                        nc.scalar.activation(sq[:], src3[:, k, lo:lo + NCH],
                                             AF.Square)
                        nc.tensor.matmul(s2_ps[:], ones_col[:], sq[:],
                                         start=(k == 0), stop=(k == CK - 1))
                    mu_row = sbp.tile([1, NCH], F32, tag="murow", bufs=1)
                    nc.scalar.activation(mu_row[:], mu_ps[:], AF.Copy,
                                         scale=1.0 / C)
                    musq = sbp.tile([1, NCH], F32, tag="musq", bufs=1)
                    nc.scalar.activation(musq[:], mu_ps[:], AF.Square,
                                         scale=1.0 / C)
                    sig = sbp.tile([1, NCH], F32, tag="sig", bufs=1)
                    nc.vector.scalar_tensor_tensor(
                        sig[:], s2_ps[:], 1.0 / C, musq[:],
                        op0=ALU.mult, op1=ALU.subtract)
                    nc.scalar.activation(sig[:], sig[:], AF.Sqrt,
                                         bias=eps_col[0:1])
                    scr = sbp.tile([1, NCH], F32, tag="scr", bufs=1)
                    nc.vector.reciprocal_approx_accurate(sig[:], sig[:],
                                                         scr[:])
                    rr = sbp.tile([1, NCH], F32R, tag="rr", bufs=2,
                                  name="rr")[:]
                    mur = sbp.tile([1, NCH], F32R, tag="mur", bufs=2,
                                   name="mur")[:]
                    nc.scalar.activation(rr, sig[:], AF.Copy)
                    nc.scalar.activation(mur, mu_row[:], AF.Copy)
                    for row, dstb in ((rr, rb), (mur, mub)):
                        b_ps = pp.tile([P, NCH], F32, tag="ps", bufs=6,
                                       name="b_ps")
                        nc.tensor.matmul(b_ps[:], ones_row[:], row,
                                         start=True, stop=True)
                        nc.vector.tensor_copy(dstb[:, lo:lo + NCH], b_ps[:])
                    if dst3 is not None:
                        for k in range(CK):
                            nc.vector.tensor_sub(dst3[:, k, lo:lo + NCH],
                                                 src3[:, k, lo:lo + NCH],
                                                 mub[:, lo:lo + NCH])
                            nc.vector.tensor_mul(dst3[:, k, lo:lo + NCH],
                                                 dst3[:, k, lo:lo + NCH],
                                                 rb[:, lo:lo + NCH])

            # ===== phase A: LN1 + QKV + scores (K via DRAM round trip) =====
            with tc.tile_pool(name="h1", bufs=1) as h1:
                va_all = h1.tile([P, S, C], BF16)     # v, token-partitioned
                ar_all = h1.tile([P, S, TQ], BF16)    # exp(att), key-part.
                with (
                    tc.tile_pool(name="px", bufs=1) as px,
                    tc.tile_pool(name="ppa", bufs=1, space="PSUM") as ppa,
                ):
                    qT = px.tile([P, CK, TQ], BF16)
                    xt = px.tile([P, CK, T], BF16)
                    # prefetch the first k-weight block ahead of x
                    wblk0 = px.tile([P, C], BF16, tag="wqk", bufs=2,
                                    name="wblk0")
                    nc.scalar.dma_start(wblk0[:], w1qk[CK])
                    for q4 in range(T // NCH):
                        for k in range(CK):
                            eng = nc.sync if k % 2 == 0 else nc.scalar
                            eng.dma_start(
                                xt[:, k, q4 * NCH:(q4 + 1) * NCH],
                                xT3[:, k, q4 * NCH:(q4 + 1) * NCH])
                    ln_stats(px, ppa, xt, T, r_b, mu_b, xt)

                    # k projection (all T tokens) -> ktd (DRAM, bf16)
                    for j in range(CK):
                        if j == 0:
                            wblk = wblk0
                        else:
                            wblk = px.tile([P, C], BF16, tag="wqk", bufs=2,
                                           name="wblk")
                            nc.scalar.dma_start(wblk[:], w1qk[CK + j])
                        for sub in range(T // NCH):
                            o_ps = ppa.tile([P, NCH], F32, tag="ps", bufs=6,
                                            name="o_ps")
                            for k in range(CK):
                                nc.tensor.matmul(
                                    o_ps[:], wblk[:, k * P:(k + 1) * P],
                                    xt[:, k, sub * NCH:(sub + 1) * NCH],
                                    start=(k == 0), stop=(k == CK - 1))
                            kev = px.tile([P, NCH], BF16, tag="kev", bufs=3)
                            nc.vector.tensor_scalar(
                                kev[:], o_ps[:], c1k_t[:, j:j + 1], None,
                                op0=ALU.add)
                            nc.gpsimd.dma_start(
                                ktd[j, :, sub * NCH:(sub + 1) * NCH], kev[:])
                    # q projection (own TQ tokens only) -> qT (SBUF)
                    for j in range(CK):
                        wblk = px.tile([P, C], BF16, tag="wqk", bufs=2,
                                       name="wblk")
                        nc.scalar.dma_start(wblk[:], w1qk[j])
                        for sub in range(TQ // NCH):
                            o_ps = ppa.tile([P, NCH], F32, tag="ps", bufs=6,
                                            name="o_ps")
                            for k in range(CK):
                                nc.tensor.matmul(
                                    o_ps[:], wblk[:, k * P:(k + 1) * P],
                                    xt[:, k, sub * NCH:(sub + 1) * NCH],
                                    start=(k == 0), stop=(k == CK - 1))
                            nc.vector.tensor_scalar(
                                qT[:, j, sub * NCH:(sub + 1) * NCH],
                                o_ps[:], c1q_t[:, j:j + 1], None,
                                op0=ALU.add)
                    # v projection (all T tokens, natural layout) -> SBUF
                    for cc in range(C // NCH):
                        w1vh = px.tile([P, CK, NCH], BF16, tag="w1vh",
                                       bufs=1)
                        for k in range(CK):
                            nc.scalar.dma_start(
                                w1vh[:, k, :],
                                w1v[k, :, cc * NCH:(cc + 1) * NCH])
                        for sl in range(S):
                            v_ps = ppa.tile([P, NCH], F32, tag="ps", bufs=6,
                                            name="v_ps")
                            for k in range(CK):
                                nc.tensor.matmul(
                                    v_ps[:], xt[:, k, sl * P:(sl + 1) * P],
                                    w1vh[:, k, :],
                                    start=(k == 0), stop=(k == CK - 1))
                            nc.vector.tensor_copy(
                                va_all[:, sl, cc * NCH:(cc + 1) * NCH],
                                v_ps[:])
                    # attention scores + exp + denominator accumulation
                    sums_ps = [ppa.tile([1, NCH], F32, tag="sums", bufs=2,
                                        name="sums")
                               for _ in range(TQ // NCH)]
                    for sl in range(S):
                        kt = px.tile([P, CK, P], BF16, tag="kt", bufs=4)
                        nc.scalar.dma_start(
                            kt[:], ktd[:, :, sl * P:(sl + 1) * P]
                            .rearrange("j p s -> p j s"))
                        for sub in range(TQ // NCH):
                            a_ps = ppa.tile([P, NCH], F32, tag="ps", bufs=6,
                                            name="a_ps")
                            for k in range(CK):
                                nc.tensor.matmul(
                                    a_ps[:], kt[:, k, :],
                                    qT[:, k, sub * NCH:(sub + 1) * NCH],
                                    start=(k == 0), stop=(k == CK - 1))
                            ae = ar_all[:, sl, sub * NCH:(sub + 1) * NCH]
                            nc.scalar.activation(ae, a_ps[:], AF.Exp,
                                                 scale=ATT_SCALE)
                            nc.tensor.matmul(sums_ps[sub][:], ones_col[:],
                                             ae, start=(sl == 0),
                                             stop=(sl == S - 1))
                    # softmax denominators -> reciprocal broadcast
                    for sub in range(TQ // NCH):
                        srow = px.tile([1, NCH], F32, tag="murow", bufs=1,
                                       name="srow")
                        nc.scalar.activation(srow[:], sums_ps[sub][:],
                                             AF.Copy)
                        scr2 = px.tile([1, NCH], F32, tag="scr", bufs=1,
                                       name="scr2")
                        nc.vector.reciprocal_approx_accurate(srow[:],
                                                             srow[:],
                                                             scr2[:])
                        srr = px.tile([1, NCH], F32R, tag="rr", bufs=2,
                                      name="srr")[:]
                        nc.scalar.activation(srr, srow[:], AF.Copy)
                        rb_ps = ppa.tile([P, NCH], F32, tag="ps", bufs=6,
                                         name="rb_ps")
                        nc.tensor.matmul(rb_ps[:], ones_row[:], srr,
                                         start=True, stop=True)
                        nc.vector.tensor_copy(
                            recip_b[:, sub * NCH:(sub + 1) * NCH], rb_ps[:])

                # ===== phase C: y = softmax @ v, proj, residual =====
                with (
                    tc.tile_pool(name="pc", bufs=1) as pc,
                    tc.tile_pool(name="ppy", bufs=1, space="PSUM") as ppy,
                ):
                    wp_sb = pc.tile([P, CK, C], BF16)
                    for j in range(CK):
                        nc.scalar.dma_start(wp_sb[:, j, :], wp[j])
                    yT = pc.tile([P, CK, TQ], BF16)
                    for sub in range(TQ // NCH):
                        y_ps = [ppy.tile([P, NCH], F32, tag="ps", bufs=8,
                                         name="y_ps") for _ in range(CK)]
                        for s in range(S):
                            for cg in range(CK):
                                nc.tensor.matmul(
                                    y_ps[cg][:],
                                    va_all[:, s, cg * P:(cg + 1) * P],
                                    ar_all[:, s, sub * NCH:(sub + 1) * NCH],
                                    start=(s == 0), stop=(s == S - 1))
                        for cg in range(CK):
                            t1 = pc.tile([P, NCH], F32, tag="yev", bufs=3)
                            nc.vector.tensor_mul(
                                t1[:], y_ps[cg][:],
                                recip_b[:, sub * NCH:(sub + 1) * NCH])
                            nc.vector.tensor_scalar(
                                yT[:, cg, sub * NCH:(sub + 1) * NCH], t1[:],
                                c1v_t[:, cg:cg + 1], None, op0=ALU.add)
                        for j in range(CK):
                            z_ps = ppy.tile([P, NCH], F32, tag="ps", bufs=8,
                                            name="z_ps")
                            for k in range(CK):
                                nc.tensor.matmul(
                                    z_ps[:], wp_sb[:, j, k * P:(k + 1) * P],
                                    yT[:, k, sub * NCH:(sub + 1) * NCH],
                                    start=(k == 0), stop=(k == CK - 1))
                            xq_t = pc.tile([P, NCH], F32, tag="xq", bufs=3)
                            nc.sync.dma_start(
                                xq_t[:], xq3[:, j, sub * NCH:(sub + 1) * NCH])
                            nc.vector.scalar_tensor_tensor(
                                x2b[:, j, sub * NCH:(sub + 1) * NCH],
                                z_ps[:], bp_t[:, j:j + 1], xq_t[:],
                                op0=ALU.add, op1=ALU.add)

            # ===== phase D: LN2 + MLP + final residual =====
            with (
                tc.tile_pool(name="pd", bufs=1) as pd,
                tc.tile_pool(name="ppd", bufs=1, space="PSUM") as ppd,
            ):
                # prefetch the first fc weight block
                wb2_0 = pd.tile([P, C], BF16, tag="wb2", bufs=3,
                                name="wb2_0")
                nc.scalar.dma_start(wb2_0[:], w2[0])
                # x2n = LN2-normalized copy of x2b (x2b kept for residual)
                x2n = pd.tile([P, CK, TQ], BF16)
                ln_stats(pd, ppd, x2b, TQ, r2_b, mu2_b, x2n)
                gel = pd.tile([P, HK, TQ], BF16)
                for jh in range(HK):
                    if jh == 0:
                        wb2 = wb2_0
                    else:
                        wb2 = pd.tile([P, C], BF16, tag="wb2", bufs=3,
                                      name="wb2")
                        nc.scalar.dma_start(wb2[:], w2[jh])
                    for sub in range(TQ // NCH):
                        m_ps = ppd.tile([P, NCH], F32, tag="ps", bufs=6,
                                        name="m_ps")
                        for k in range(CK):
                            nc.tensor.matmul(
                                m_ps[:], wb2[:, k * P:(k + 1) * P],
                                x2n[:, k, sub * NCH:(sub + 1) * NCH],
                                start=(k == 0), stop=(k == CK - 1))
                        nc.scalar.activation(
                            gel[:, jh, sub * NCH:(sub + 1) * NCH], m_ps[:],
                            AF.Gelu_apprx_tanh, bias=c2_t[:, jh:jh + 1])
                for j in range(CK):
                    wmh = pd.tile([P, H], BF16, tag="wmh", bufs=2)
                    nc.scalar.dma_start(wmh[:], wm[j])
                    for sub in range(TQ // NCH):
                        o_ps = ppd.tile([P, NCH], F32, tag="ps", bufs=6,
                                        name="o_ps")
                        for kk in range(HK):
                            nc.tensor.matmul(
                                o_ps[:], wmh[:, kk * P:(kk + 1) * P],
                                gel[:, kk, sub * NCH:(sub + 1) * NCH],
                                start=(kk == 0), stop=(kk == HK - 1))
                        o_sb = pd.tile([P, NCH], F32, tag="osb", bufs=3)
                        nc.vector.scalar_tensor_tensor(
                            o_sb[:], o_ps[:], bm_t[:, j:j + 1],
                            x2b[:, j, sub * NCH:(sub + 1) * NCH],
                            op0=ALU.add, op1=ALU.add)
                        nc.gpsimd.dma_start(
                            out_t[j * P:(j + 1) * P,
                                  sub * NCH:(sub + 1) * NCH], o_sb[:])
    nc.finalize()
    return nc


_prog = None


def _get_prog():
    global _prog
    if _prog is None:
        _prog = _build()
    return _prog


def _pack_weights(ln1_g, ln1_b, w_attn, b_attn, w_proj, b_proj,
                  ln2_g, ln2_b, w_fc, b_fc, w_mlp_proj, b_mlp_proj):
    f = np.float32
    W1 = (ln1_g[:, None] * w_attn).astype(BFNP)          # [C, 3C] bf16
    W1f = W1.astype(f)
    c1 = (ln1_b @ W1f + b_attn).astype(f)                # [3C]
    w1qk = np.ascontiguousarray(
        W1[:, :2 * C].reshape(CK, P, 2 * CK, P).transpose(2, 1, 0, 3)
        .reshape(2 * CK, P, C))
    w1v = np.ascontiguousarray(W1[:, 2 * C:].reshape(CK, P, C))
    wp_t = np.ascontiguousarray(
        w_proj.astype(BFNP).reshape(CK, P, CK, P).transpose(2, 1, 0, 3)
        .reshape(CK, P, C))
    W2 = (ln2_g[:, None] * w_fc).astype(BFNP)            # [C, H] bf16
    W2f = W2.astype(f)
    c2v = (ln2_b @ W2f + b_fc).astype(f)                 # [H]
    w2_t = np.ascontiguousarray(
        W2.reshape(CK, P, HK, P).transpose(2, 1, 0, 3).reshape(HK, P, C))
    wm_t = np.ascontiguousarray(
        w_mlp_proj.astype(BFNP).reshape(HK, P, CK, P).transpose(2, 1, 0, 3)
        .reshape(CK, P, H))
    return {
        "w1qk": w1qk,
        "w1v": w1v,
        "wp": wp_t,
        "w2": w2_t,
        "wm": wm_t,
        "c1q": np.ascontiguousarray(c1[:C].reshape(CK, P)),
        "c1k": np.ascontiguousarray(c1[C:2 * C].reshape(CK, P)),
        "c1v": np.ascontiguousarray(c1[2 * C:].reshape(CK, P)),
        "bp": np.ascontiguousarray(b_proj.astype(f).reshape(CK, P)),
        "c2": np.ascontiguousarray(c2v.reshape(HK, P)),
        "bm": np.ascontiguousarray(b_mlp_proj.astype(f).reshape(CK, P)),
        "onc": np.ones((P, 1), BFNP),
        "onr": np.ones((1, P), f),
    }


def kernel(x, ln1_g, ln1_b, w_attn, b_attn, w_proj, b_proj,
           ln2_g, ln2_b, w_fc, b_fc, w_mlp_proj, b_mlp_proj,
           _trace=False):
    x = np.asarray(x, np.float32)
    shared = _pack_weights(
        np.asarray(ln1_g, np.float32), np.asarray(ln1_b, np.float32),
        np.asarray(w_attn, np.float32), np.asarray(b_attn, np.float32),
        np.asarray(w_proj, np.float32), np.asarray(b_proj, np.float32),
        np.asarray(ln2_g, np.float32), np.asarray(ln2_b, np.float32),
        np.asarray(w_fc, np.float32), np.asarray(b_fc, np.float32),
        np.asarray(w_mlp_proj, np.float32), np.asarray(b_mlp_proj,
                                                       np.float32))

    in_maps = []
    for core in range(N_CORES):
        b, h = core // 2, core % 2
        xf = x[b].T                                     # [C, T]
        own = xf[:, h * TQ:(h + 1) * TQ]
        other = xf[:, (1 - h) * TQ:(2 - h) * TQ]
        xTb = np.ascontiguousarray(
            np.concatenate([own, other], axis=1)).astype(BFNP)
        xq32 = np.ascontiguousarray(own)
        in_maps.append({"xTb": xTb, "xq32": xq32, **shared})

    nc = _get_prog()
    res = run_bass_kernel_spmd(nc, in_maps, list(range(N_CORES)),
                               trace=_trace)
    out = np.empty_like(x)
    for core in range(N_CORES):
        b, h = core // 2, core % 2
        out[b, h * TQ:(h + 1) * TQ] = res.results[core]["out_t"].T
    if _trace:
        kernel._last_exec_time_ns = res.exec_time_ns
        kernel._last_profile = res.profile_json
    return out
